# revision 13
# baseline (speedup 1.0000x reference)
"""Bass/Trainium2 kernel for nn_HardAndLayer.

Reference computation:
    out[o] = AND_i ( x[i] OR NOT w[o,i] )   , w in {0.0, 1.0}, x bool
           = NOT any_i ( w[o,i] AND NOT x[i] )

Strategy:
  - Weights are binary -> bit-pack 32 weights per uint32 word on the host.
    Full W [8192, 8192] f32 (256 MB) becomes [8192, 256] uint32 (8 MB).
  - Shard rows across 8 cores (1024 rows / core = 1 MB packed per core).
    The bit-packed NOT-x vector is replicated across the 128 partitions and
    prepended to the per-core weight buffer, so a single DMA stream feeds
    everything.
  - Per core: 3 chunked DMAs (issued on both HWDGE engines: sync + scalar),
    8 VectorE bitwise_and ops (one per 128-row tile), 8 ScalarE
    activation(Copy)+accumulate ops producing the per-row sum of AND words.
    A row has a violation iff its sum > 0.
  - Host: out[row] = (sum == 0).
"""

import sys

if "/opt/trn_rl_repo" not in sys.path:
    sys.path.insert(0, "/opt/trn_rl_repo")

import numpy as np

import concourse.bacc as bacc
import concourse.mybir as mybir
import concourse.tile as tile
from concourse.bass_utils import run_bass_kernel_spmd

OUT, IN = 8192, 8192
NCORES = 8
P = 128                 # SBUF partitions
NW = IN // 32           # uint32 words per row = 256
RPC = OUT // NCORES     # rows per core = 1024
NT = RPC // P           # 128-row tiles per core = 8
TOT = NW * (NT + 1)     # words per partition incl. leading nx block = 2304

# chunk boundaries in tiles; chunk 0 additionally carries the leading nx block
CHUNKS = [(0, 1), (1, 3), (3, 5), (5, 7), (7, 8)]

_cached = {}


def _build_module():
    nc = bacc.Bacc(None, enable_partition_id=False, enable_asserts=False)
    wx = nc.dram_tensor("wx", [P, TOT], mybir.dt.uint32, kind="ExternalInput")
    out = nc.dram_tensor("out", [P, NT], mybir.dt.float32, kind="ExternalOutput")

    with tile.TileContext(nc) as tc:
        with tc.tile_pool(name="sbuf", bufs=1) as pool:
            # chunk tiles; chunk 0 carries [nx | tile0]
            ctiles = []
            for ci, (ta, tb) in enumerate(CHUNKS):
                lo = ta * NW if ci else 0
                hi = (tb + 1) * NW
                ck = pool.tile([P, hi - lo], mybir.dt.uint32, tag=f"c{ci}")
                eng = nc.sync if ci % 2 == 0 else nc.scalar
                eng.dma_start(ck[:], wx[:, lo:hi])
                ctiles.append((ck, lo))

            nxs = ctiles[0][0][:, 0:NW]
            res = pool.tile([P, NT], mybir.dt.float32)
            # Reduce engine split: ScalarE (activation+accum) takes the early
            # tiles so its slower per-op chain starts first; VectorE
            # tensor_reduce takes the late tiles. Both chains overlap with
            # the VectorE bitwise_and stream.
            N_ACT = 4
            for t in range(NT):
                ci = next(i for i, (ta, tb) in enumerate(CHUNKS) if ta <= t < tb)
                ck, lo = ctiles[ci]
                off = (t + 1) * NW - lo
                sl = ck[:, off : off + NW]
                nc.vector.tensor_tensor(
                    out=sl, in0=sl, in1=nxs, op=mybir.AluOpType.bitwise_and
                )
                if t < N_ACT:
                    nc.scalar.activation(
                        out=sl,
                        in_=sl,
                        func=mybir.ActivationFunctionType.Copy,
                        accum_out=res[:, t : t + 1],
                    )
                else:
                    nc.vector.tensor_reduce(
                        out=res[:, t : t + 1],
                        in_=sl,
                        axis=mybir.AxisListType.X,
                        op=mybir.AluOpType.max,
                    )

            nc.sync.dma_start(out[:], res[:])
    nc.compile()
    return nc


def _pack_bits(bool2d: np.ndarray) -> np.ndarray:
    """[N, 8192] bool -> [N, 256] uint32 (consistent bit order)."""
    u8 = np.packbits(bool2d, axis=-1, bitorder="little")
    return u8.view(np.uint32)


def kernel(weights: np.ndarray, x: np.ndarray, **run_kwargs):
    wbits = _pack_bits(np.asarray(weights) != 0)                # [8192, 256]
    nxbits = _pack_bits((~np.asarray(x, dtype=bool))[None, :])  # [1, 256]
    nx_rep = np.broadcast_to(nxbits, (P, NW))

    in_maps = []
    for c in range(NCORES):
        wr = (
            wbits[c * RPC : (c + 1) * RPC]
            .reshape(NT, P, NW)
            .transpose(1, 0, 2)
            .reshape(P, NT * NW)
        )
        in_maps.append({"wx": np.ascontiguousarray(np.concatenate([nx_rep, wr], axis=1))})

    if "nc" not in _cached:
        _cached["nc"] = _build_module()
    nc = _cached["nc"]

    r = run_bass_kernel_spmd(nc, in_maps, core_ids=list(range(NCORES)), **run_kwargs)

    outs = []
    for c in range(NCORES):
        m = r.results[c]["out"]            # [P, NT] f32, m[p, t] = sum of AND words
        outs.append(m.T.reshape(RPC))      # row t*128+p within core
    sums = np.concatenate(outs)            # [8192]
    result = sums == 0.0
    if run_kwargs:
        return result, r
    return result
